# revision 12
# baseline (speedup 1.0000x reference)
"""Two-block transformer encoder (B=4, S=2048, D=256, H=8, DFF=1024) on 8
Trainium2 NeuronCores.

Sharding: core c -> batch b = c//2, sequence half = c%2 (1024 tokens owned).
Weights are replicated. Block 1 computes K/V over the full sequence (the
full x[b] is an input) and Q/FFN/LN over the owned half only. Between the
two blocks the halves of the block-1 output are exchanged within each core
pair with per-chunk AllGathers; block 2 computes K/V from the gathered
activations and everything else from the locally kept half.

Layout: activations are feature-major ([d, token]) in SBUF so projections
are plain matmuls with pre-transposed weights. Attention scores land in
[key, query] orientation (stationary = K^T slice, contraction = head dim
32, four heads packed into the PE rows). exp() runs on the scalar engine
straight out of PSUM with the 1/sqrt(dk) scale fused, emitting fp8e4
probabilities. P@V runs as fp8 DoubleRow matmuls (two key chunks per
instruction) with two heads column-packed into one 128-partition PSUM
tile; a ones column in V yields the softmax denominator, which is
reciprocated on a compact [8, 512] tile and broadcast across partitions
via a DRAM bounce. The own-token range is processed in two 512-query
chunks: each chunk's output projection + LN + FFN + LN (and, after block
1, its half of the exchange) overlap the next chunk's attention.

The host pre-swaps each core's own token half to columns 0:1024 so the
compiled program is identical across cores (pure SPMD); key order inside
attention is permutation invariant so gathered K/V ordering needs no
fixup.
"""

import numpy as np

import concourse.bass as bass
import concourse.mybir as mybir
import concourse.tile as tile
from concourse.bass_utils import run_bass_kernel_spmd

# ---------------------------------------------------------------- constants
B, S, D, H, DK, DFF = 4, 2048, 256, 8, 32, 1024
NCORES = 8
TOWN = S // 2  # tokens owned per core
QT = 512  # query chunk
NQT = TOWN // QT  # 2
KC = 128  # key chunk (PE contraction)
NKC = S // KC  # 16
NKCP = NKC // 2  # 8 key-chunk pairs (fp8 DoubleRow)
DC = D // 128  # 2 feature chunks
FC = DFF // 128  # 8 ff chunks
NPAIR = H // 2  # 4 head pairs (h, h+4)
EPS = 1e-5
SCALE = float(1.0 / np.sqrt(np.float32(DK)))
F32 = mybir.dt.float32
F32R = mybir.dt.float32r
BF16 = mybir.dt.bfloat16
FP8 = mybir.dt.float8e4
GROUPS = [[0, 1], [2, 3], [4, 5], [6, 7]]
VW = 64  # v columns per head in fp8 stationary (32 v + ones col + pad)


def _legalize_multiwaits(nc):
    """This container's walrus supports one semaphore wait per instruction;
    split multi-wait instructions into prefix EventSemaphore waits."""
    import json

    orig = nc.to_json_bytes

    def patched():
        j = json.loads(orig())
        n = 0
        for fn in j.get("functions", []):
            for bb in fn.get("blocks", []):
                out = []
                for ins in bb.get("instructions", []):
                    si = ins.get("sync_info") or {}
                    waits = si.get("on_wait") or []
                    if len(waits) > 1:
                        for w in waits[:-1]:
                            n += 1
                            out.append({
                                "engine": ins["engine"],
                                "ins": [],
                                "name": f"I-mwsplit-{n}",
                                "opcode": "EventSemaphore",
                                "outs": [],
                                "sync_info": {"on_update": [], "on_wait": [w]},
                            })
                        si["on_wait"] = [waits[-1]]
                    out.append(ins)
                bb["instructions"] = out
        return json.dumps(j).encode()

    nc.to_json_bytes = patched
    return nc


def _mm(nc, out, lhsT, rhs, **kw):
    nc.tensor.matmul(out, lhsT, rhs, **kw)


def build(debug=False):
    from contextlib import ExitStack

    nc = bass.Bass(num_devices=NCORES)

    xt_in = nc.dram_tensor("xt", [DC, 128, S], F32R, kind="ExternalInput")
    wd = {}
    for bi in range(2):
        for nm in ("wq", "wk", "wv", "wo"):
            wd[f"{nm}{bi}"] = nc.dram_tensor(f"{nm}{bi}", [DC, 128, D], F32R, kind="ExternalInput")
        wd[f"wf1{bi}"] = nc.dram_tensor(f"wf1{bi}", [DC, 128, DFF], F32R, kind="ExternalInput")
        wd[f"wf2{bi}"] = nc.dram_tensor(f"wf2{bi}", [FC, 128, D], F32R, kind="ExternalInput")
        for nm in ("ga", "ba", "gb", "bb"):
            wd[f"{nm}{bi}"] = nc.dram_tensor(f"{nm}{bi}", [DC, 128, 1], F32, kind="ExternalInput")
    out_t = nc.dram_tensor("out_t", [DC, 128, TOWN], F32R, kind="ExternalOutput")
    xh_d = nc.dram_tensor("xh_d", [NQT, DC, 128, QT], F32R)
    xg_d = nc.dram_tensor("xg_d", [NQT, 2, DC, 128, QT], F32R)
    bc_d = nc.dram_tensor("bc_d", [2, NQT, 8, QT], F32R)

    with tile.TileContext(nc) as tc, ExitStack() as top:
        top.enter_context(nc.allow_low_precision(
            reason="fp8 attention probabilities/V; matmul accumulation fp32"))
        persist = top.enter_context(tc.tile_pool(name="persist", bufs=1))

        ones32 = persist.tile([128, 32], F32R, tag="ones32", name="ones32")
        nc.vector.memset(ones32.bitcast(F32), 1.0)
        scale_row = persist.tile([1, 128], F32R, tag="scale_row", name="scale_row")
        nc.vector.memset(scale_row.bitcast(F32), 1.0 / D)
        eps128 = persist.tile([128, 1], F32, tag="eps128", name="eps128")
        nc.vector.memset(eps128, EPS)
        consts = {"ones32": ones32, "scale_row": scale_row, "eps128": eps128}

        # ---- block input first (unblocks QKV quickly), then weights
        xt = [persist.tile([128, S], F32R, tag=f"xt{i}", name=f"xt{i}") for i in range(DC)]
        for i in range(DC):
            nc.sync.dma_start(xt[i], xt_in[i])
        W = {}
        for bi in range(2):
            for nm, chunks, width in (
                ("wq", DC, D), ("wk", DC, D), ("wv", DC, D), ("wo", DC, D),
                ("wf1", DC, DFF), ("wf2", FC, D),
            ):
                t = persist.tile([128, chunks, width], F32R, tag=f"{nm}{bi}", name=f"{nm}{bi}")
                for c in range(chunks):
                    nc.sync.dma_start(t[:, c, :], wd[f"{nm}{bi}"][c])
                W[f"{nm}{bi}"] = t
            for nm in ("ga", "ba", "gb", "bb"):
                t = persist.tile([128, DC, 1], F32, tag=f"{nm}{bi}", name=f"{nm}{bi}")
                for c in range(DC):
                    nc.sync.dma_start(t[:, c, :], wd[f"{nm}{bi}"][c])
                W[f"{nm}{bi}"] = t

        # fp8 token-major V, [keys=128, kc-in-pair=2, head=8, VW]; ones+pad
        # columns (32:64) are set once and only cols 0:32 are rewritten per
        # block, so the den column stays valid for both blocks.
        vtok8 = [persist.tile([128, 2, H, VW], FP8, tag=f"vt{k}", name=f"vt{k}")
                 for k in range(NKCP)]
        for k in range(NKCP):
            nc.vector.memset(vtok8[k][:, :, :, DK:VW], 1.0)

        x2own = [[persist.tile([128, TOWN], F32R, tag=f"x2own{bi}_{i}", name=f"x2own{bi}_{i}")
                  for i in range(DC)] for bi in range(2)]

        for bi in range(2):
            blk = top.enter_context(tc.tile_pool(name=f"blk{bi}", bufs=1))
            src_q = [xt[i][:, 0:TOWN] for i in range(DC)] if bi == 0 else \
                    [x2own[0][i][:] for i in range(DC)]
            src_kv = xt

            # ============ QKV projections =============================
            kT = [[blk.tile([128, QT], BF16, tag=f"kT{i}_{st}", name=f"kT{i}_{st}")
                   for st in range(S // QT)] for i in range(DC)]
            qT = [[blk.tile([128, QT], BF16, tag=f"qT{i}_{qt}", name=f"qT{i}_{qt}")
                   for qt in range(NQT)] for i in range(DC)]
            with ExitStack() as qst:
                psA = qst.enter_context(tc.tile_pool(name=f"psA{bi}", bufs=3, space="PSUM"))
                psV = qst.enter_context(tc.tile_pool(name=f"psV{bi}", bufs=1, space="PSUM"))
                # q^T, own tokens only
                for oc in range(DC):
                    for qt in range(NQT):
                        ps = psA.tile([128, QT], F32, tag="qkv", name="qkv")
                        for ic in range(DC):
                            _mm(nc, ps[:], W[f"wq{bi}"][:, ic, oc * 128:(oc + 1) * 128],
                                src_q[ic][:, qt * QT:(qt + 1) * QT],
                                start=(ic == 0), stop=(ic == DC - 1))
                        nc.vector.tensor_scalar_max(qT[oc][qt][:], ps[:], 0.0)
                # k^T over the full sequence
                for oc in range(DC):
                    for st in range(S // QT):
                        ps = psA.tile([128, QT], F32, tag="qkv", name="qkv")
                        for ic in range(DC):
                            _mm(nc, ps[:], W[f"wk{bi}"][:, ic, oc * 128:(oc + 1) * 128],
                                src_kv[ic][:, st * QT:(st + 1) * QT],
                                start=(ic == 0), stop=(ic == DC - 1))
                        if st % 2 == 0:
                            nc.vector.tensor_scalar_max(kT[oc][st][:], ps[:], 0.0)
                        else:
                            nc.scalar.activation(kT[oc][st][:], ps[:],
                                                 mybir.ActivationFunctionType.Relu)
                # token-major fp8 V with the head dim strided into vtok8
                for kc in range(NKC):
                    ps = psV.tile([128, D], F32, tag="vtok", name="vtok")
                    for ic in range(DC):
                        _mm(nc, ps[:], src_kv[ic][:, kc * 128:(kc + 1) * 128],
                            W[f"wv{bi}"][:, ic, :],
                            start=(ic == 0), stop=(ic == DC - 1))
                    nc.vector.tensor_scalar_max(
                        vtok8[kc // 2][:, kc % 2, :, 0:DK],
                        ps[:].rearrange("p (h k) -> p h k", h=H), 0.0)

            # ============ attention + post pipeline per query chunk ====
            for qt in range(NQT):
                with ExitStack() as ast:
                    atmp = ast.enter_context(tc.tile_pool(name=f"at{bi}_{qt}", bufs=1))
                    p8p = ast.enter_context(tc.tile_pool(name=f"p8{bi}_{qt}", bufs=3))
                    psB = ast.enter_context(tc.tile_pool(name=f"psB{bi}_{qt}", bufs=2, space="PSUM"))
                    psPV = ast.enter_context(tc.tile_pool(name=f"psPV{bi}_{qt}", bufs=2, space="PSUM"))

                    ot = [atmp.tile([128, QT], F32R, tag=f"ot{g}", name=f"ot{g}")
                          for g in range(DC)]
                    den_c = atmp.tile([128, 2, QT], F32, tag="den", name="den")
                    for pair in range(NPAIR):
                        pv = [psPV.tile([64, QT], F32, tag="pv", name="pv")
                              for _ in range(2)]
                        for kcp in range(NKCP):
                            for hi in range(2):  # h = pair (g=0), pair+4 (g=1)
                                p8 = p8p.tile([128, 2, QT], FP8, tag="p8", name="p8")
                                for kci in range(2):
                                    kc = 2 * kcp + kci
                                    sc = psB.tile([128, QT], F32, tag="sc", name="sc")
                                    _mm(nc, sc[:],
                                        kT[hi][kc // 4][32 * pair:32 * pair + 32,
                                                        (kc % 4) * 128:(kc % 4 + 1) * 128],
                                        qT[hi][qt][32 * pair:32 * pair + 32, :],
                                        start=True, stop=True,
                                        tile_position=(32 * pair, 0),
                                        skip_group_check=True)
                                    nc.scalar.activation(
                                        p8[:, kci, :], sc[:],
                                        mybir.ActivationFunctionType.Exp, scale=SCALE)
                                _mm(nc, pv[hi][:],
                                    vtok8[kcp][:, :, 4 * hi + pair, :],
                                    p8[:],
                                    start=(kcp == 0), stop=(kcp == NKCP - 1),
                                    perf_mode=mybir.MatmulPerfMode.DoubleRow,
                                    skip_group_check=True)
                        # extract o rows; stage den rows (32-aligned
                        # partitions) and ship them to DRAM for broadcast
                        for hi in range(2):
                            nc.vector.tensor_copy(
                                ot[hi][32 * pair:32 * pair + 32, :],
                                pv[hi][0:32, :])
                            drow = 32 * (2 * (pair % 2) + hi)
                            dseg = pair // 2
                            nc.vector.tensor_copy(
                                den_c[drow:drow + 1, dseg, :],
                                pv[hi][DK:DK + 1, :])
                            nc.sync.dma_start(
                                bc_d[bi, qt, 2 * pair + hi],
                                den_c[drow:drow + 1, dseg, :].bitcast(F32R))
                    # normalize: broadcast-read den across partitions, one
                    # reciprocal per head group, multiply into ot
                    rb = [atmp.tile([128, QT], F32R, tag=f"rb{g}", name=f"rb{g}")
                          for g in range(DC)]
                    for pair in range(NPAIR):
                        for hi in range(2):
                            row = bc_d[bi, qt, 2 * pair + hi]
                            bcast = bass.AP(tensor=row.tensor, offset=row.offset,
                                            ap=[[0, 32], *[list(dd) for dd in row.ap]])
                            nc.gpsimd.dma_start(
                                rb[hi][32 * pair:32 * pair + 32, :], bcast)
                    for g in range(DC):
                        nc.vector.reciprocal(rb[g][:].bitcast(F32), rb[g][:].bitcast(F32))
                        nc.vector.tensor_mul(ot[g][:], ot[g][:], rb[g][:])

                    # ======== Wo proj + residual + LN1 (this chunk) ====
                    resid1 = [src_q[i][:, qt * QT:(qt + 1) * QT] for i in range(DC)]
                    x1 = [atmp.tile([128, QT], F32R, tag=f"x1_{i}", name=f"x1_{i}")
                          for i in range(DC)]
                    with ExitStack() as pst:
                        ptmp = pst.enter_context(tc.tile_pool(name=f"pt{bi}_{qt}", bufs=1))
                        psP = pst.enter_context(tc.tile_pool(name=f"psP{bi}_{qt}", bufs=1, space="PSUM"))
                        psS = pst.enter_context(tc.tile_pool(name=f"psS{bi}_{qt}", bufs=1, space="PSUM"))
                        psC = pst.enter_context(tc.tile_pool(name=f"psC{bi}_{qt}", bufs=1, space="PSUM"))
                        self_ln(nc, tc, ptmp, psP, psS, psC, W, f"ga{bi}", f"ba{bi}",
                                ot, resid1, [x1[i][:] for i in range(DC)],
                                consts, proj=(W[f"wo{bi}"], DC))

                        # ======== FFN + residual + LN2 =================
                        hT = ptmp.tile([128, FC, QT], F32R, tag="hT", name="hT")
                        for fc in range(FC):
                            ps = psP.tile([128, QT], F32, tag="proj", name="proj")
                            for ic in range(DC):
                                _mm(nc, ps[:], W[f"wf1{bi}"][:, ic, fc * 128:(fc + 1) * 128],
                                    x1[ic][:], start=(ic == 0), stop=(ic == DC - 1))
                            if fc % 2 == 0:
                                nc.scalar.activation(
                                    hT[:, fc, :], ps[:],
                                    mybir.ActivationFunctionType.Relu)
                            else:
                                nc.vector.tensor_scalar_max(hT[:, fc, :], ps[:], 0.0)
                        x2sl = [x2own[bi][i][:, qt * QT:(qt + 1) * QT] for i in range(DC)]
                        self_ln(nc, tc, ptmp, psP, psS, psC, W, f"gb{bi}", f"bb{bi}",
                                [hT[:, fc, :] for fc in range(FC)],
                                [x1[i][:] for i in range(DC)], x2sl,
                                consts, proj=(W[f"wf2{bi}"], FC))

                # exchange this chunk (block 0) / write output (block 1)
                if bi == 0:
                    for i in range(DC):
                        nc.sync.dma_start(xh_d[qt, i], x2own[0][i][:, qt * QT:(qt + 1) * QT])
                    nc.gpsimd.collective_compute(
                        "AllGather", mybir.AluOpType.bypass,
                        replica_groups=GROUPS,
                        ins=[xh_d[qt].flatten()], outs=[xg_d[qt].flatten()])
                    for i in range(DC):
                        for r in range(2):
                            nc.sync.dma_start(
                                xt[i][:, r * TOWN + qt * QT:r * TOWN + (qt + 1) * QT],
                                xg_d[qt, r, i])
                else:
                    for i in range(DC):
                        nc.sync.dma_start(out_t[i][:, qt * QT:(qt + 1) * QT],
                                          x2own[1][i][:, qt * QT:(qt + 1) * QT])

    return _legalize_multiwaits(nc)


def self_ln(nc, tc, tmp, psP, psS, psC, W, gkey, bkey, moving, resid, out_aps,
            consts, proj):
    """Project the `moving` chunks with `proj`, relu, add `resid`, layer-norm
    with (gamma=W[gkey], beta=W[bkey]) -> out_aps. One 512-query chunk.

    Per-token LN stats are computed with ones-matmuls (feature axis lives on
    partitions, sum and sum-of-squares packed into one PSUM bank) and
    broadcast back across partitions with K=1 matmuls against a constant 1/D
    row, so the whole chain stays on wide ops.
    """
    wt, nch = proj
    ones32 = consts["ones32"]
    scale_row = consts["scale_row"]

    y = [tmp.tile([128, QT], F32R, tag=f"y{i}", name=f"y{i}", bufs=2) for i in range(DC)]
    for oc in range(DC):
        ps = psP.tile([128, QT], F32, tag="proj", name="proj")
        for ic in range(nch):
            _mm(nc, ps[:], wt[:, ic, oc * 128:(oc + 1) * 128], moving[ic],
                start=(ic == 0), stop=(ic == nch - 1))
        # y = relu(ps) + resid
        nc.vector.scalar_tensor_tensor(
            y[oc][:], ps[:], 0.0, resid[oc],
            op0=mybir.AluOpType.max, op1=mybir.AluOpType.add)
    # stats: per-token sum and sum-of-squares via ones-matmuls
    ssum = psS.tile([32, QT], F32, tag="ssum", name="ssum")
    ssq = psS.tile([32, QT], F32, tag="ssq", name="ssq")
    for oc in range(DC):
        ysq = tmp.tile([128, QT], F32R, tag="ysq", name="ysq", bufs=2)
        nc.scalar.activation(ysq[:], y[oc][:],
                             mybir.ActivationFunctionType.Square)
        _mm(nc, ssum[:], ones32, y[oc][:],
            start=(oc == 0), stop=(oc == DC - 1), skip_group_check=True)
        _mm(nc, ssq[:], ones32, ysq[:],
            start=(oc == 0), stop=(oc == DC - 1), skip_group_check=True)
    srows = tmp.tile([1, 2, QT], F32R, tag="srows", name="srows", bufs=2)
    nc.vector.tensor_copy(srows[:, 0, :], ssum[0:1, :])
    nc.vector.tensor_copy(srows[:, 1, :], ssq[0:1, :])
    # broadcast mean and mean-square across partitions (K=1 matmuls against
    # a 1/D row, folding in the division); one bank used twice
    mb = psC.tile([128, QT], F32, tag="bc", name="bc")
    _mm(nc, mb[:], scale_row, srows[:, 0, :], start=True, stop=True,
        skip_group_check=True)
    msb = tmp.tile([128, QT], F32, tag="msb", name="msb", bufs=2)
    nc.vector.tensor_copy(msb[:], mb[:])
    m2 = psC.tile([128, QT], F32, tag="bc", name="bc")
    _mm(nc, m2[:], scale_row, srows[:, 1, :], start=True, stop=True,
        skip_group_check=True)
    # var = m2 - mu^2 ; rstd = 1/sqrt(var + eps)
    vb = tmp.tile([128, QT], F32, tag="vb", name="vb", bufs=2)
    nc.vector.tensor_mul(vb[:], msb[:], msb[:])
    nc.vector.tensor_sub(vb[:], m2[:], vb[:])
    rb = tmp.tile([128, QT], F32, tag="rbln", name="rbln", bufs=2)
    nc.scalar.activation(rb[:], vb[:],
                         mybir.ActivationFunctionType.Sqrt,
                         bias=consts["eps128"])
    nc.vector.reciprocal(rb[:], rb[:])
    for oc in range(DC):
        t = tmp.tile([128, QT], F32, tag="t", name="t", bufs=2)
        nc.vector.tensor_sub(t[:], y[oc][:], msb[:])
        nc.vector.scalar_tensor_tensor(
            t[:], t[:], W[gkey][:, oc, :], rb[:],
            op0=mybir.AluOpType.mult, op1=mybir.AluOpType.mult)
        nc.vector.tensor_scalar_add(out_aps[oc], t[:], W[bkey][:, oc, :])


def _install_profile_hook():
    """Expose the axon NTFF profiling hook that bass_utils expects (the
    agent image's antenv lacks axon_hooks). Only used when tracing."""
    import sys as _sys
    import types as _types

    if "antenv.axon_hooks" in _sys.modules:
        return
    _sys.path.insert(0, "/root/.axon_site")
    try:
        from trn_agent_boot.trn_boot import _ntff_profile_via_ctypes
        hook = _ntff_profile_via_ctypes("/opt/axon/libaxon_pjrt.so")
    except Exception:
        hook = None
    mod = _types.ModuleType("antenv.axon_hooks")
    mod.get_axon_ntff_profile_hook = lambda: hook
    mod.set_axon_ntff_profile_hook = lambda h: None
    _sys.modules["antenv.axon_hooks"] = mod


# ---------------------------------------------------------------- host side
_NC_CACHE = {}


def _get_nc(debug=False):
    if debug not in _NC_CACHE:
        _NC_CACHE[debug] = build(debug)
    return _NC_CACHE[debug]


def _prep_inputs(x, weights):
    """Per-core input dicts. x: (B, S, D) fp32. weights: dict of np arrays."""
    in_maps = []
    wmats = {}
    for bi, (q, k, v, o, f1, f2) in enumerate(
        (("W11", "W12", "W13", "W14", "Wf11", "Wf21"),
         ("W21", "W22", "W23", "W24", "Wf12", "Wf22"))):
        wmats[f"wq{bi}"] = np.ascontiguousarray(
            weights[q].T.reshape(DC, 128, D))
        wmats[f"wk{bi}"] = np.ascontiguousarray(
            weights[k].T.reshape(DC, 128, D))
        wmats[f"wv{bi}"] = np.ascontiguousarray(
            weights[v].T.reshape(DC, 128, D))
        wmats[f"wo{bi}"] = np.ascontiguousarray(
            weights[o].T.reshape(DC, 128, D))
        wmats[f"wf1{bi}"] = np.ascontiguousarray(
            weights[f1].T.reshape(DC, 128, DFF))
        wmats[f"wf2{bi}"] = np.ascontiguousarray(
            weights[f2].T.reshape(FC, 128, D))
    for bi, (g1, b1, g2, b2) in enumerate(
        (("g1", "b1", "g2", "b2"), ("g3", "b3", "g4", "b4"))):
        wmats[f"ga{bi}"] = np.ascontiguousarray(
            weights[g1].reshape(DC, 128, 1))
        wmats[f"ba{bi}"] = np.ascontiguousarray(
            weights[b1].reshape(DC, 128, 1))
        wmats[f"gb{bi}"] = np.ascontiguousarray(
            weights[g2].reshape(DC, 128, 1))
        wmats[f"bb{bi}"] = np.ascontiguousarray(
            weights[b2].reshape(DC, 128, 1))
    for c in range(NCORES):
        b, half = c // 2, c % 2
        xb = x[b]  # (S, D)
        own = xb[half * TOWN:(half + 1) * TOWN]
        other = xb[(1 - half) * TOWN:(2 - half) * TOWN]
        xcore = np.concatenate([own, other], axis=0)  # own tokens first
        xt = np.ascontiguousarray(xcore.T.reshape(DC, 128, S))
        m = {"xt": xt}
        m.update(wmats)
        in_maps.append(m)
    return in_maps


def kernel(x, W11, W12, W13, W14, W21, W22, W23, W24,
           Wf11, Wf21, Wf12, Wf22,
           g1, b1, g2, b2, g3, b3, g4, b4, _debug=False, _trace=False):
    weights = dict(W11=W11, W12=W12, W13=W13, W14=W14,
                   W21=W21, W22=W22, W23=W23, W24=W24,
                   Wf11=Wf11, Wf21=Wf21, Wf12=Wf12, Wf22=Wf22,
                   g1=g1, b1=b1, g2=g2, b2=b2, g3=g3, b3=b3, g4=g4, b4=b4)
    weights = {k: np.asarray(v, dtype=np.float32) for k, v in weights.items()}
    x = np.asarray(x, dtype=np.float32)
    if _trace:
        _install_profile_hook()
    nc = _get_nc(False)
    in_maps = _prep_inputs(x, weights)
    res = run_bass_kernel_spmd(nc, in_maps, core_ids=list(range(NCORES)),
                               trace=_trace)
    out = np.empty((B, S, D), dtype=np.float32)
    for c in range(NCORES):
        b, half = c // 2, c % 2
        ot = res.results[c]["out_t"].reshape(D, TOWN)
        out[b, half * TOWN:(half + 1) * TOWN] = ot.T
    if _debug or _trace:
        kernel.last_result = res
    return out


# revision 13
# speedup vs baseline: 1.1155x; 1.1155x over previous
"""Two-block transformer encoder (B=4, S=2048, D=256, H=8, DFF=1024) on 8
Trainium2 NeuronCores.

Sharding: core c -> batch b = c//2, sequence half = c%2 (1024 tokens owned).
Weights are replicated. Block 1 computes K/V over the full sequence (the
full x[b] is an input) and Q/FFN/LN over the owned half only. Between the
two blocks the halves of the block-1 output are exchanged within each core
pair with per-chunk AllGathers; block 2 computes K/V from the gathered
activations and everything else from the locally kept half.

Layout: activations are feature-major ([d, token]) in SBUF so projections
are plain matmuls with pre-transposed weights. Attention scores land in
[key, query] orientation (stationary = K^T slice, contraction = head dim
32, four heads packed into the PE rows). exp() runs on the scalar engine
straight out of PSUM with the 1/sqrt(dk) scale fused, emitting fp8e4
probabilities. P@V runs as fp8 DoubleRow matmuls (two key chunks per
instruction) with two heads column-packed into one 128-partition PSUM
tile; a ones column in V yields the softmax denominator, which is
reciprocated on a compact [8, 512] tile and broadcast across partitions
via a DRAM bounce. The own-token range is processed in two 512-query
chunks: each chunk's output projection + LN + FFN + LN (and, after block
1, its half of the exchange) overlap the next chunk's attention.

The host pre-swaps each core's own token half to columns 0:1024 so the
compiled program is identical across cores (pure SPMD); key order inside
attention is permutation invariant so gathered K/V ordering needs no
fixup.
"""

import numpy as np

import concourse.bass as bass
import concourse.mybir as mybir
import concourse.tile as tile
from concourse.bass_utils import run_bass_kernel_spmd

# ---------------------------------------------------------------- constants
B, S, D, H, DK, DFF = 4, 2048, 256, 8, 32, 1024
NCORES = 8
TOWN = S // 2  # tokens owned per core
QT = 512  # query chunk
NQT = TOWN // QT  # 2
KC = 128  # key chunk (PE contraction)
NKC = S // KC  # 16
NKCP = NKC // 2  # 8 key-chunk pairs (fp8 DoubleRow)
DC = D // 128  # 2 feature chunks
FC = DFF // 128  # 8 ff chunks
NPAIR = H // 2  # 4 head pairs (h, h+4)
EPS = 1e-5
SCALE = float(1.0 / np.sqrt(np.float32(DK)))
F32 = mybir.dt.float32
F32R = mybir.dt.float32r
BF16 = mybir.dt.bfloat16
FP8 = mybir.dt.float8e4
GROUPS = [[0, 1], [2, 3], [4, 5], [6, 7]]
VW = 64  # v columns per head in fp8 stationary (32 v + ones col + pad)


def _legalize_multiwaits(nc):
    """This container's walrus supports one semaphore wait per instruction;
    split multi-wait instructions into prefix EventSemaphore waits."""
    import json

    orig = nc.to_json_bytes

    def patched():
        j = json.loads(orig())
        n = 0
        for fn in j.get("functions", []):
            for bb in fn.get("blocks", []):
                out = []
                for ins in bb.get("instructions", []):
                    si = ins.get("sync_info") or {}
                    waits = si.get("on_wait") or []
                    if len(waits) > 1:
                        for w in waits[:-1]:
                            n += 1
                            out.append({
                                "engine": ins["engine"],
                                "ins": [],
                                "name": f"I-mwsplit-{n}",
                                "opcode": "EventSemaphore",
                                "outs": [],
                                "sync_info": {"on_update": [], "on_wait": [w]},
                            })
                        si["on_wait"] = [waits[-1]]
                    out.append(ins)
                bb["instructions"] = out
        return json.dumps(j).encode()

    nc.to_json_bytes = patched
    return nc


def _mm(nc, out, lhsT, rhs, **kw):
    nc.tensor.matmul(out, lhsT, rhs, **kw)


def build(debug=False):
    from contextlib import ExitStack

    nc = bass.Bass(num_devices=NCORES)

    xt_in = nc.dram_tensor("xt", [DC, 128, S], F32R, kind="ExternalInput")
    wd = {}
    for bi in range(2):
        for nm in ("wq", "wk", "wv", "wo"):
            wd[f"{nm}{bi}"] = nc.dram_tensor(f"{nm}{bi}", [DC, 128, D], F32R, kind="ExternalInput")
        wd[f"wf1{bi}"] = nc.dram_tensor(f"wf1{bi}", [DC, 128, DFF], F32R, kind="ExternalInput")
        wd[f"wf2{bi}"] = nc.dram_tensor(f"wf2{bi}", [FC, 128, D], F32R, kind="ExternalInput")
        for nm in ("ga", "ba", "gb", "bb"):
            wd[f"{nm}{bi}"] = nc.dram_tensor(f"{nm}{bi}", [DC, 128, 1], F32, kind="ExternalInput")
    out_t = nc.dram_tensor("out_t", [DC, 128, TOWN], F32R, kind="ExternalOutput")
    xh_d = nc.dram_tensor("xh_d", [NQT, DC, 128, QT], F32R)
    xg_d = nc.dram_tensor("xg_d", [NQT, 2, DC, 128, QT], F32R)
    bc_d = nc.dram_tensor("bc_d", [2, NQT, 8, QT], F32R)

    with tile.TileContext(nc) as tc, ExitStack() as top:
        top.enter_context(nc.allow_low_precision(
            reason="fp8 attention probabilities/V; matmul accumulation fp32"))
        persist = top.enter_context(tc.tile_pool(name="persist", bufs=1))

        ones32 = persist.tile([128, 32], F32R, tag="ones32", name="ones32")
        nc.vector.memset(ones32.bitcast(F32), 1.0)
        scale_row = persist.tile([1, 128], F32R, tag="scale_row", name="scale_row")
        nc.vector.memset(scale_row.bitcast(F32), 1.0 / D)
        eps128 = persist.tile([128, 1], F32, tag="eps128", name="eps128")
        nc.vector.memset(eps128, EPS)
        consts = {"ones32": ones32, "scale_row": scale_row, "eps128": eps128}

        # ---- block input first (unblocks QKV quickly), then weights
        xt = [persist.tile([128, S], F32R, tag=f"xt{i}", name=f"xt{i}") for i in range(DC)]
        for i in range(DC):
            nc.sync.dma_start(xt[i], xt_in[i])
        W = {}
        for bi in range(2):
            for nm, chunks, width in (
                ("wq", DC, D), ("wk", DC, D), ("wv", DC, D), ("wo", DC, D),
                ("wf1", DC, DFF), ("wf2", FC, D),
            ):
                t = persist.tile([128, chunks, width], F32R, tag=f"{nm}{bi}", name=f"{nm}{bi}")
                for c in range(chunks):
                    nc.sync.dma_start(t[:, c, :], wd[f"{nm}{bi}"][c])
                W[f"{nm}{bi}"] = t
            for nm in ("ga", "ba", "gb", "bb"):
                t = persist.tile([128, DC, 1], F32, tag=f"{nm}{bi}", name=f"{nm}{bi}")
                for c in range(DC):
                    nc.sync.dma_start(t[:, c, :], wd[f"{nm}{bi}"][c])
                W[f"{nm}{bi}"] = t

        # fp8 token-major V, [keys=128, kc-in-pair=2, head=8, VW]; ones+pad
        # columns (32:64) are set once and only cols 0:32 are rewritten per
        # block, so the den column stays valid for both blocks.
        vtok8 = [persist.tile([128, 2, H, VW], FP8, tag=f"vt{k}", name=f"vt{k}")
                 for k in range(NKCP)]
        for k in range(NKCP):
            nc.vector.memset(vtok8[k][:, :, :, DK:VW], 1.0)

        x2own = [[persist.tile([128, TOWN], F32R, tag=f"x2own{bi}_{i}", name=f"x2own{bi}_{i}")
                  for i in range(DC)] for bi in range(2)]

        for bi in range(2):
            blk = top.enter_context(tc.tile_pool(name=f"blk{bi}", bufs=1))
            src_q = [xt[i][:, 0:TOWN] for i in range(DC)] if bi == 0 else \
                    [x2own[0][i][:] for i in range(DC)]
            src_kv = xt

            # ============ QKV projections =============================
            kT = [[blk.tile([128, QT], BF16, tag=f"kT{i}_{st}", name=f"kT{i}_{st}")
                   for st in range(S // QT)] for i in range(DC)]
            qT = [[blk.tile([128, QT], BF16, tag=f"qT{i}_{qt}", name=f"qT{i}_{qt}")
                   for qt in range(NQT)] for i in range(DC)]
            with ExitStack() as qst:
                psA = qst.enter_context(tc.tile_pool(name=f"psA{bi}", bufs=3, space="PSUM"))
                psV = qst.enter_context(tc.tile_pool(name=f"psV{bi}", bufs=1, space="PSUM"))
                # q^T, own tokens only
                for oc in range(DC):
                    for qt in range(NQT):
                        ps = psA.tile([128, QT], F32, tag="qkv", name="qkv")
                        for ic in range(DC):
                            _mm(nc, ps[:], W[f"wq{bi}"][:, ic, oc * 128:(oc + 1) * 128],
                                src_q[ic][:, qt * QT:(qt + 1) * QT],
                                start=(ic == 0), stop=(ic == DC - 1))
                        nc.vector.tensor_scalar_max(qT[oc][qt][:], ps[:], 0.0)
                # k^T over the full sequence
                for oc in range(DC):
                    for st in range(S // QT):
                        ps = psA.tile([128, QT], F32, tag="qkv", name="qkv")
                        for ic in range(DC):
                            _mm(nc, ps[:], W[f"wk{bi}"][:, ic, oc * 128:(oc + 1) * 128],
                                src_kv[ic][:, st * QT:(st + 1) * QT],
                                start=(ic == 0), stop=(ic == DC - 1))
                        nc.vector.tensor_scalar_max(kT[oc][st][:], ps[:], 0.0)
                # token-major fp8 V with the head dim strided into vtok8
                for kc in range(NKC):
                    ps = psV.tile([128, D], F32, tag="vtok", name="vtok")
                    for ic in range(DC):
                        _mm(nc, ps[:], src_kv[ic][:, kc * 128:(kc + 1) * 128],
                            W[f"wv{bi}"][:, ic, :],
                            start=(ic == 0), stop=(ic == DC - 1))
                    nc.vector.tensor_scalar_max(
                        vtok8[kc // 2][:, kc % 2, :, 0:DK],
                        ps[:].rearrange("p (h k) -> p h k", h=H), 0.0)

            # ============ attention + post pipeline per query chunk ====
            for qt in range(NQT):
                with ExitStack() as ast:
                    atmp = ast.enter_context(tc.tile_pool(name=f"at{bi}_{qt}", bufs=1))
                    p8p = ast.enter_context(tc.tile_pool(name=f"p8{bi}_{qt}", bufs=3))
                    psB = ast.enter_context(tc.tile_pool(name=f"psB{bi}_{qt}", bufs=2, space="PSUM"))
                    psPV = ast.enter_context(tc.tile_pool(name=f"psPV{bi}_{qt}", bufs=2, space="PSUM"))

                    ot = [atmp.tile([128, QT], F32R, tag=f"ot{g}", name=f"ot{g}")
                          for g in range(DC)]
                    den_c = atmp.tile([128, 2, QT], F32, tag="den", name="den")
                    for pair in range(NPAIR):
                        pv = [psPV.tile([64, QT], F32, tag="pv", name="pv")
                              for _ in range(2)]
                        pend = None  # (kcp, p8 pair) awaiting PV issue
                        for kcp in range(NKCP):
                            p8s = []
                            for hi in range(2):  # h = pair (g=0), pair+4 (g=1)
                                p8 = p8p.tile([128, 2, QT], FP8, tag="p8", name="p8")
                                for kci in range(2):
                                    kc = 2 * kcp + kci
                                    sc = psB.tile([128, QT], F32, tag="sc", name="sc")
                                    _mm(nc, sc[:],
                                        kT[hi][kc // 4][32 * pair:32 * pair + 32,
                                                        (kc % 4) * 128:(kc % 4 + 1) * 128],
                                        qT[hi][qt][32 * pair:32 * pair + 32, :],
                                        start=True, stop=True,
                                        tile_position=(32 * pair, 0),
                                        skip_group_check=True)
                                    nc.scalar.activation(
                                        p8[:, kci, :], sc[:],
                                        mybir.ActivationFunctionType.Exp, scale=SCALE)
                                p8s.append(p8)
                            if pend is not None:
                                pk, pp = pend
                                for hi in range(2):
                                    _mm(nc, pv[hi][:],
                                        vtok8[pk][:, :, 4 * hi + pair, :],
                                        pp[hi][:],
                                        start=(pk == 0), stop=False,
                                        perf_mode=mybir.MatmulPerfMode.DoubleRow,
                                        skip_group_check=True)
                            pend = (kcp, p8s)
                        pk, pp = pend
                        for hi in range(2):
                            _mm(nc, pv[hi][:],
                                vtok8[pk][:, :, 4 * hi + pair, :],
                                pp[hi][:],
                                start=False, stop=True,
                                perf_mode=mybir.MatmulPerfMode.DoubleRow,
                                skip_group_check=True)
                        # extract o rows; stage den rows (32-aligned
                        # partitions) and ship them to DRAM for broadcast
                        for hi in range(2):
                            nc.vector.tensor_copy(
                                ot[hi][32 * pair:32 * pair + 32, :],
                                pv[hi][0:32, :])
                            drow = 32 * (2 * (pair % 2) + hi)
                            dseg = pair // 2
                            nc.vector.tensor_copy(
                                den_c[drow:drow + 1, dseg, :],
                                pv[hi][DK:DK + 1, :])
                            nc.sync.dma_start(
                                bc_d[bi, qt, 2 * pair + hi],
                                den_c[drow:drow + 1, dseg, :].bitcast(F32R))
                    # normalize: broadcast-read den across partitions, one
                    # reciprocal per head group, multiply into ot
                    rb = [atmp.tile([128, QT], F32R, tag=f"rb{g}", name=f"rb{g}")
                          for g in range(DC)]
                    for pair in range(NPAIR):
                        for hi in range(2):
                            row = bc_d[bi, qt, 2 * pair + hi]
                            bcast = bass.AP(tensor=row.tensor, offset=row.offset,
                                            ap=[[0, 32], *[list(dd) for dd in row.ap]])
                            nc.gpsimd.dma_start(
                                rb[hi][32 * pair:32 * pair + 32, :], bcast)
                    for g in range(DC):
                        nc.vector.reciprocal(rb[g][:].bitcast(F32), rb[g][:].bitcast(F32))
                        nc.vector.tensor_mul(ot[g][:], ot[g][:], rb[g][:])

                    # ======== Wo proj + residual + LN1 (this chunk) ====
                    resid1 = [src_q[i][:, qt * QT:(qt + 1) * QT] for i in range(DC)]
                    x1 = [atmp.tile([128, QT], F32R, tag=f"x1_{i}", name=f"x1_{i}")
                          for i in range(DC)]
                    with ExitStack() as pst:
                        ptmp = pst.enter_context(tc.tile_pool(name=f"pt{bi}_{qt}", bufs=1))
                        psP = pst.enter_context(tc.tile_pool(name=f"psP{bi}_{qt}", bufs=1, space="PSUM"))
                        psS = pst.enter_context(tc.tile_pool(name=f"psS{bi}_{qt}", bufs=1, space="PSUM"))
                        psC = pst.enter_context(tc.tile_pool(name=f"psC{bi}_{qt}", bufs=1, space="PSUM"))
                        self_ln(nc, tc, ptmp, psP, psS, psC, W, f"ga{bi}", f"ba{bi}",
                                ot, resid1, [x1[i][:] for i in range(DC)],
                                consts, proj=(W[f"wo{bi}"], DC))

                        # ======== FFN + residual + LN2 =================
                        hT = ptmp.tile([128, FC, QT], F32R, tag="hT", name="hT")
                        for fc in range(FC):
                            ps = psP.tile([128, QT], F32, tag="proj", name="proj")
                            for ic in range(DC):
                                _mm(nc, ps[:], W[f"wf1{bi}"][:, ic, fc * 128:(fc + 1) * 128],
                                    x1[ic][:], start=(ic == 0), stop=(ic == DC - 1))
                            nc.vector.tensor_scalar_max(hT[:, fc, :], ps[:], 0.0)
                        x2sl = [x2own[bi][i][:, qt * QT:(qt + 1) * QT] for i in range(DC)]
                        self_ln(nc, tc, ptmp, psP, psS, psC, W, f"gb{bi}", f"bb{bi}",
                                [hT[:, fc, :] for fc in range(FC)],
                                [x1[i][:] for i in range(DC)], x2sl,
                                consts, proj=(W[f"wf2{bi}"], FC))

                # exchange this chunk (block 0) / write output (block 1)
                if bi == 0:
                    for i in range(DC):
                        nc.sync.dma_start(xh_d[qt, i], x2own[0][i][:, qt * QT:(qt + 1) * QT])
                    nc.gpsimd.collective_compute(
                        "AllGather", mybir.AluOpType.bypass,
                        replica_groups=GROUPS,
                        ins=[xh_d[qt].flatten()], outs=[xg_d[qt].flatten()])
                    for i in range(DC):
                        for r in range(2):
                            nc.sync.dma_start(
                                xt[i][:, r * TOWN + qt * QT:r * TOWN + (qt + 1) * QT],
                                xg_d[qt, r, i])
                else:
                    for i in range(DC):
                        nc.sync.dma_start(out_t[i][:, qt * QT:(qt + 1) * QT],
                                          x2own[1][i][:, qt * QT:(qt + 1) * QT])

    return _legalize_multiwaits(nc)


def self_ln(nc, tc, tmp, psP, psS, psC, W, gkey, bkey, moving, resid, out_aps,
            consts, proj):
    """Project the `moving` chunks with `proj`, relu, add `resid`, layer-norm
    with (gamma=W[gkey], beta=W[bkey]) -> out_aps. One 512-query chunk.

    Per-token LN stats are computed with ones-matmuls (feature axis lives on
    partitions, sum and sum-of-squares packed into one PSUM bank) and
    broadcast back across partitions with K=1 matmuls against a constant 1/D
    row, so the whole chain stays on wide ops.
    """
    wt, nch = proj
    ones32 = consts["ones32"]
    scale_row = consts["scale_row"]

    y = [tmp.tile([128, QT], F32R, tag=f"y{i}", name=f"y{i}", bufs=2) for i in range(DC)]
    for oc in range(DC):
        ps = psP.tile([128, QT], F32, tag="proj", name="proj")
        for ic in range(nch):
            _mm(nc, ps[:], wt[:, ic, oc * 128:(oc + 1) * 128], moving[ic],
                start=(ic == 0), stop=(ic == nch - 1))
        # y = relu(ps) + resid
        nc.vector.scalar_tensor_tensor(
            y[oc][:], ps[:], 0.0, resid[oc],
            op0=mybir.AluOpType.max, op1=mybir.AluOpType.add)
    # stats: per-token sum and sum-of-squares via ones-matmuls
    ssum = psS.tile([32, QT], F32, tag="ssum", name="ssum")
    ssq = psS.tile([32, QT], F32, tag="ssq", name="ssq")
    for oc in range(DC):
        ysq = tmp.tile([128, QT], F32R, tag="ysq", name="ysq", bufs=2)
        nc.vector.tensor_mul(ysq[:], y[oc][:], y[oc][:])
        _mm(nc, ssum[:], ones32, y[oc][:],
            start=(oc == 0), stop=(oc == DC - 1), skip_group_check=True)
        _mm(nc, ssq[:], ones32, ysq[:],
            start=(oc == 0), stop=(oc == DC - 1), skip_group_check=True)
    srows = tmp.tile([1, 2, QT], F32R, tag="srows", name="srows", bufs=2)
    nc.vector.tensor_copy(srows[:, 0, :], ssum[0:1, :])
    nc.vector.tensor_copy(srows[:, 1, :], ssq[0:1, :])
    # broadcast mean and mean-square across partitions (K=1 matmuls against
    # a 1/D row, folding in the division); one bank used twice
    mb = psC.tile([128, QT], F32, tag="bc", name="bc")
    _mm(nc, mb[:], scale_row, srows[:, 0, :], start=True, stop=True,
        skip_group_check=True)
    msb = tmp.tile([128, QT], F32, tag="msb", name="msb", bufs=2)
    nc.vector.tensor_copy(msb[:], mb[:])
    m2 = psC.tile([128, QT], F32, tag="bc", name="bc")
    _mm(nc, m2[:], scale_row, srows[:, 1, :], start=True, stop=True,
        skip_group_check=True)
    # var = m2 - mu^2 ; rstd = 1/sqrt(var + eps)
    vb = tmp.tile([128, QT], F32, tag="vb", name="vb", bufs=2)
    nc.vector.tensor_mul(vb[:], msb[:], msb[:])
    nc.vector.tensor_sub(vb[:], m2[:], vb[:])
    rb = tmp.tile([128, QT], F32, tag="rbln", name="rbln", bufs=2)
    nc.scalar.activation(rb[:], vb[:],
                         mybir.ActivationFunctionType.Sqrt,
                         bias=consts["eps128"])
    nc.vector.reciprocal(rb[:], rb[:])
    for oc in range(DC):
        t = tmp.tile([128, QT], F32, tag="t", name="t", bufs=2)
        nc.vector.tensor_sub(t[:], y[oc][:], msb[:])
        nc.vector.scalar_tensor_tensor(
            t[:], t[:], W[gkey][:, oc, :], rb[:],
            op0=mybir.AluOpType.mult, op1=mybir.AluOpType.mult)
        nc.vector.tensor_scalar_add(out_aps[oc], t[:], W[bkey][:, oc, :])


def _install_profile_hook():
    """Expose the axon NTFF profiling hook that bass_utils expects (the
    agent image's antenv lacks axon_hooks). Only used when tracing."""
    import sys as _sys
    import types as _types

    if "antenv.axon_hooks" in _sys.modules:
        return
    _sys.path.insert(0, "/root/.axon_site")
    try:
        from trn_agent_boot.trn_boot import _ntff_profile_via_ctypes
        hook = _ntff_profile_via_ctypes("/opt/axon/libaxon_pjrt.so")
    except Exception:
        hook = None
    mod = _types.ModuleType("antenv.axon_hooks")
    mod.get_axon_ntff_profile_hook = lambda: hook
    mod.set_axon_ntff_profile_hook = lambda h: None
    _sys.modules["antenv.axon_hooks"] = mod


# ---------------------------------------------------------------- host side
_NC_CACHE = {}


def _get_nc(debug=False):
    if debug not in _NC_CACHE:
        _NC_CACHE[debug] = build(debug)
    return _NC_CACHE[debug]


def _prep_inputs(x, weights):
    """Per-core input dicts. x: (B, S, D) fp32. weights: dict of np arrays."""
    in_maps = []
    wmats = {}
    for bi, (q, k, v, o, f1, f2) in enumerate(
        (("W11", "W12", "W13", "W14", "Wf11", "Wf21"),
         ("W21", "W22", "W23", "W24", "Wf12", "Wf22"))):
        wmats[f"wq{bi}"] = np.ascontiguousarray(
            weights[q].T.reshape(DC, 128, D))
        wmats[f"wk{bi}"] = np.ascontiguousarray(
            weights[k].T.reshape(DC, 128, D))
        wmats[f"wv{bi}"] = np.ascontiguousarray(
            weights[v].T.reshape(DC, 128, D))
        wmats[f"wo{bi}"] = np.ascontiguousarray(
            weights[o].T.reshape(DC, 128, D))
        wmats[f"wf1{bi}"] = np.ascontiguousarray(
            weights[f1].T.reshape(DC, 128, DFF))
        wmats[f"wf2{bi}"] = np.ascontiguousarray(
            weights[f2].T.reshape(FC, 128, D))
    for bi, (g1, b1, g2, b2) in enumerate(
        (("g1", "b1", "g2", "b2"), ("g3", "b3", "g4", "b4"))):
        wmats[f"ga{bi}"] = np.ascontiguousarray(
            weights[g1].reshape(DC, 128, 1))
        wmats[f"ba{bi}"] = np.ascontiguousarray(
            weights[b1].reshape(DC, 128, 1))
        wmats[f"gb{bi}"] = np.ascontiguousarray(
            weights[g2].reshape(DC, 128, 1))
        wmats[f"bb{bi}"] = np.ascontiguousarray(
            weights[b2].reshape(DC, 128, 1))
    for c in range(NCORES):
        b, half = c // 2, c % 2
        xb = x[b]  # (S, D)
        own = xb[half * TOWN:(half + 1) * TOWN]
        other = xb[(1 - half) * TOWN:(2 - half) * TOWN]
        xcore = np.concatenate([own, other], axis=0)  # own tokens first
        xt = np.ascontiguousarray(xcore.T.reshape(DC, 128, S))
        m = {"xt": xt}
        m.update(wmats)
        in_maps.append(m)
    return in_maps


def kernel(x, W11, W12, W13, W14, W21, W22, W23, W24,
           Wf11, Wf21, Wf12, Wf22,
           g1, b1, g2, b2, g3, b3, g4, b4, _debug=False, _trace=False):
    weights = dict(W11=W11, W12=W12, W13=W13, W14=W14,
                   W21=W21, W22=W22, W23=W23, W24=W24,
                   Wf11=Wf11, Wf21=Wf21, Wf12=Wf12, Wf22=Wf22,
                   g1=g1, b1=b1, g2=g2, b2=b2, g3=g3, b3=b3, g4=g4, b4=b4)
    weights = {k: np.asarray(v, dtype=np.float32) for k, v in weights.items()}
    x = np.asarray(x, dtype=np.float32)
    if _trace:
        _install_profile_hook()
    nc = _get_nc(False)
    in_maps = _prep_inputs(x, weights)
    res = run_bass_kernel_spmd(nc, in_maps, core_ids=list(range(NCORES)),
                               trace=_trace)
    out = np.empty((B, S, D), dtype=np.float32)
    for c in range(NCORES):
        b, half = c // 2, c % 2
        ot = res.results[c]["out_t"].reshape(D, TOWN)
        out[b, half * TOWN:(half + 1) * TOWN] = ot.T
    if _debug or _trace:
        kernel.last_result = res
    return out
